# revision 1
# baseline (speedup 1.0000x reference)
"""INT8-dynamic-quant GEMM (per-token act quant, per-channel weight quant,
int8 GEMM, dequant) on 8 trn2 NeuronCores.

Math: the int8 values (|v| <= 127) are carried exactly in bf16; bf16 matmuls
with fp32 PSUM accumulation reproduce the int32 GEMM exactly (partial sums
stay far below 2^24). Rounding uses the +1.5*2^23 magic-number trick on the
DVE (exact IEEE fp32), which is round-to-nearest-even like jnp.round.

Per-core layout (core c -> m-group c//2 of 4, n-group c%2 of 2):
  x slice [2048, 4096], w slice [5504, 4096], out slice [2048, 5504].

Pipeline per core (single Tile graph). Emission order is pipelined so the
GEMM starts as soon as x-quant + the first w chunk are done; w-quant of
chunk j+1 is emitted between chunk j's transposes and matmuls so it
overlaps. Bulk fp32 input reads + out writes go on the scalar HWDGE queue,
GEMM-critical transposes on the sync queue (two independent DMA FIFOs).

  1) x-quant: 16 row-tiles [128, K]: DVE abs-max -> scales; scale+round on
     DVE -> bf16 HBM scratch (2 half tensors); sx also routed to a flat
     [Mc] DRAM row (strided DMA) for the rank-1 scale matmuls.
  2) w-quant: 43 row-tiles -> 11 chunk scratch tensors [<=512, K] bf16;
     per-chunk sw flat DRAM row likewise.
  3) GEMM: for each m-half (xqt [128, 32, MH] bf16 resident via 32 xbar
     transpose-DMAs), stream w chunks transposed [128, 32, 512] (pass 0:
     xbar transposes + plain copy-back to DRAM; pass 1: one plain 4MB DMA),
     32-matmul PSUM accumulation per (m-tile, chunk). Dequant: PE rank-1
     matmul psum_scale[m,n] = sx[m]*sw[n] (lhsT=sx row slice, rhs=sw row,
     K=1), then one DVE multiply on eviction - matching the reference's
     (x_scales * w_scales) association exactly.
"""

import contextlib
import math

import numpy as np

import concourse.bass as bass
from concourse import bacc
import concourse.mybir as mybir
import concourse.tile as tile
from concourse import bass_utils

F32 = mybir.dt.float32
BF16 = mybir.dt.bfloat16
P = 128
MAGIC = 1.5 * 2**23  # fp32 round-to-nearest-even forcing constant
EPS = 1e-5

# Full problem
M, K, N = 8192, 4096, 11008
MG, NG = 4, 2  # core grid: 4 m-groups x 2 n-groups
MC, NC = M // MG, N // NG  # per-core 2048 x 5504


def quant_body(tc, x_ap, w_ap, out_ap, Mc, Nc, Kd, chunk=512, m_halves=2):
    """Emit the whole per-core kernel into TileContext tc."""
    nc = tc.nc
    KT = Kd // P  # k tiles
    MT = Mc // P  # m tiles (quant granularity)
    NT = Nc // P  # n tiles
    MH = Mc // m_halves  # m rows per half
    MHT = MH // P  # m tiles per half
    CNT = chunk // P  # n tiles per full chunk
    n_chunks = math.ceil(Nc / chunk)
    ch_w = [min(chunk, Nc - j * chunk) for j in range(n_chunks)]

    est = contextlib.ExitStack()
    with est:
        dram = est.enter_context(tc.tile_pool(name="dram", bufs=1, space="DRAM"))
        qin = est.enter_context(tc.tile_pool(name="qin", bufs=2))
        qout = est.enter_context(tc.tile_pool(name="qout", bufs=2))
        small = est.enter_context(tc.tile_pool(name="small", bufs=4))
        scl = est.enter_context(tc.tile_pool(name="scl", bufs=1))
        rowp = est.enter_context(tc.tile_pool(name="rowp", bufs=2))
        swrep = est.enter_context(tc.tile_pool(name="swrep", bufs=2))
        xqtp = est.enter_context(tc.tile_pool(name="xqtp", bufs=1))
        wqtp = est.enter_context(tc.tile_pool(name="wqtp", bufs=8))
        obp = est.enter_context(tc.tile_pool(name="obp", bufs=2))
        psum = est.enter_context(tc.tile_pool(name="psum", bufs=6, space="PSUM"))
        pssc = est.enter_context(tc.tile_pool(name="pssc", bufs=2, space="PSUM"))

        # ---- persistent scratch ----
        QPH = 2 if MH >= 2 * P and (MH // P) % 2 == 0 else 1  # quarters/half
        MQ = MH // QPH
        xq_q = [
            dram.tile([MQ, Kd], BF16, tag=f"xq{q}", name=f"xq{q}")
            for q in range(QPH * m_halves)
        ]
        wq_ch = [
            dram.tile([ch_w[j], Kd], BF16, tag=f"wq{j}", name=f"wq{j}")
            for j in range(n_chunks)
        ]
        wqt_ch = [
            dram.tile([P, KT, ch_w[j]], BF16, tag=f"wqtd{j}", name=f"wqtd{j}")
            for j in range(n_chunks)
        ]
        swsc_ch = [
            dram.tile([ch_w[j]], F32, tag=f"swsc{j}", name=f"swsc{j}")
            for j in range(n_chunks)
        ]
        sx_striped = scl.tile([P, MT], F32, tag="sx")
        sw_striped = scl.tile([P, NT], F32, tag="sw")
        ones_row = scl.tile([1, P], F32, tag="ones")
        nc.vector.memset(ones_row, 1.0)

        def quant_tile(src_rows, dst_rows, scale_col):
            """[128, Kd] fp32 rows -> bf16 int values + per-row scale col."""
            tin = qin.tile([P, Kd], F32, tag="qin")
            nc.gpsimd.dma_start(tin, src_rows)
            mx = small.tile([P, 1], F32, tag="mx")
            nc.vector.tensor_reduce(
                mx, tin, axis=mybir.AxisListType.X, op=mybir.AluOpType.max,
                apply_absolute_value=True,
            )
            nc.vector.tensor_scalar_max(mx, mx, EPS)
            # scale = mx/127 (matches reference up to 1 ulp)
            nc.vector.tensor_scalar_mul(scale_col, mx, 1.0 / 127.0)
            inv = small.tile([P, 1], F32, tag="inv")
            nc.vector.reciprocal(inv, scale_col)
            # t = x*inv + MAGIC ; q = t - MAGIC  (RNE round, output bf16)
            # Both on DVE: exact IEEE fp32 (ACT's internal precision differs).
            nc.vector.tensor_scalar(
                tin, tin, inv, MAGIC,
                mybir.AluOpType.mult, mybir.AluOpType.add,
            )
            q = qout.tile([P, Kd], BF16, tag="qout")
            nc.vector.tensor_scalar(
                q, tin, MAGIC, None, mybir.AluOpType.subtract,
            )
            nc.gpsimd.dma_start(dst_rows, q)

        def emit_w_quant(j):
            nt = ch_w[j] // P
            for t in range(nt):
                jj = j * CNT + t
                quant_tile(
                    w_ap[jj * P : (jj + 1) * P, :],
                    wq_ch[j][t * P : (t + 1) * P, :],
                    sw_striped[:, jj : jj + 1],
                )
            # striped [128, nt] -> flat DRAM [ch_w] in n-order
            nc.gpsimd.dma_start(
                swsc_ch[j].rearrange("(o p) -> p o", p=P),
                sw_striped[:, j * CNT : j * CNT + nt],
            )

        # ---- phase 1: x quant, per quarter ----
        MQT = MQ // P
        def emit_x_quant_q(q):
            for r in range(MQT):
                i = q * MQT + r
                quant_tile(
                    x_ap[i * P : (i + 1) * P, :],
                    xq_q[q][r * P : (r + 1) * P, :],
                    sx_striped[:, i : i + 1],
                )

        # interleave x quarters with the first w chunks so the first GEMM
        # chunk's inputs are ready as early as possible
        wq_emitted = 0
        for q in range(QPH * m_halves):
            emit_x_quant_q(q)
            if wq_emitted < min(2, n_chunks):
                emit_w_quant(wq_emitted)
                wq_emitted += 1
        while wq_emitted < min(4, n_chunks):
            emit_w_quant(wq_emitted)
            wq_emitted += 1

        # ---- phase 2/3 interleaved: GEMM + remaining w quant ----
        for mh in range(m_halves):
            xqt = xqtp.tile([P, KT, MH], BF16, tag="xqt")
            for qq in range(QPH):
                q = mh * QPH + qq
                for k in range(KT):
                    nc.sync.dma_start_transpose(
                        xqt[:, k, qq * MQ : (qq + 1) * MQ],
                        xq_q[q][:, k * P : (k + 1) * P],
                    )
            for j in range(n_chunks):
                wd = ch_w[j]
                KQ = 4 if KT % 4 == 0 else 1  # k-quarters per chunk
                KTQ = KT // KQ
                wqt_q = []
                for kq in range(KQ):
                    wq_tile = wqtp.tile(
                        [P, KTQ, chunk], BF16, tag="wqt", name=f"wqt_{mh}_{j}_{kq}"
                    )
                    if mh == 0:
                        for kk in range(KTQ):
                            k = kq * KTQ + kk
                            nc.sync.dma_start_transpose(
                                wq_tile[:, kk, :wd],
                                wq_ch[j][:, k * P : (k + 1) * P],
                            )
                        # save transposed copy; later halves read it plainly
                        nc.gpsimd.dma_start(
                            wqt_ch[j][:, kq * KTQ : (kq + 1) * KTQ, :],
                            wq_tile[:, :, :wd],
                        )
                    else:
                        nc.sync.dma_start(
                            wq_tile[:, :, :wd],
                            wqt_ch[j][:, kq * KTQ : (kq + 1) * KTQ, :],
                        )
                    wqt_q.append(wq_tile)
                # overlap: quantize 4 chunks ahead of the GEMM
                if mh == 0 and j + 4 < n_chunks and j + 4 >= 4:
                    emit_w_quant(j + 4)
                swr = rowp.tile([1, chunk], F32, tag="swrow")
                nc.scalar.dma_start(swr[0:1, :wd], swsc_ch[j][None, :])
                scps = pssc.tile([P, chunk], F32, tag="scps")
                nc.tensor.matmul(
                    scps[:, :wd], lhsT=ones_row, rhs=swr[0:1, :wd],
                    start=True, stop=True,
                )
                swb = swrep.tile([P, chunk], F32, tag="swrep")
                nc.scalar.copy(swb[:, :wd], scps[:, :wd])
                IB = 3  # i-block size (psum banks)
                for ib0 in range(0, MHT, IB):
                    ii = list(range(ib0, min(ib0 + IB, MHT)))
                    pss = {}
                    for i in ii:
                        pss[i] = psum.tile(
                            [P, chunk], F32, tag="ps", name=f"ps{i}"
                        )
                    for k in range(KT):
                        wq_tile = wqt_q[k // KTQ]
                        for i in ii:
                            nc.tensor.matmul(
                                pss[i][:, :wd],
                                lhsT=xqt[:, k, i * P : (i + 1) * P],
                                rhs=wq_tile[:, k % KTQ, :wd],
                                start=(k == 0),
                                stop=(k == KT - 1),
                            )
                    ob = obp.tile([P, len(ii), chunk], F32, tag="ob")
                    for ix, i in enumerate(ii):
                        gi = mh * MHT + i
                        nc.scalar.activation(
                            ob[:, ix, :wd], pss[i][:, :wd],
                            mybir.ActivationFunctionType.Copy,
                            scale=sx_striped[:, gi : gi + 1],
                        )
                        nc.vector.tensor_tensor(
                            ob[:, ix, :wd], ob[:, ix, :wd], swb[:, :wd],
                            mybir.AluOpType.mult,
                        )
                    g0 = (mh * MHT + ii[0]) * P
                    nc.scalar.dma_start(
                        out_ap[g0 : g0 + len(ii) * P,
                               j * chunk : j * chunk + wd].rearrange(
                            "(a p) c -> p a c", p=P
                        ),
                        ob[:, :, :wd],
                    )


def build_nc(Mc=MC, Nc=NC, Kd=K, chunk=512, m_halves=2):
    nc = bacc.Bacc("TRN2", target_bir_lowering=False, debug=False, num_devices=8)
    x_d = nc.dram_tensor("x", [Mc, Kd], F32, kind="ExternalInput")
    w_d = nc.dram_tensor("w", [Nc, Kd], F32, kind="ExternalInput")
    out_d = nc.dram_tensor("out", [Mc, Nc], F32, kind="ExternalOutput")
    with tile.TileContext(nc) as tc:
        quant_body(tc, x_d.ap(), w_d.ap(), out_d.ap(), Mc, Nc, Kd, chunk, m_halves)
    nc.compile()
    return nc


_NC_CACHE = {}


def kernel(**inputs):
    x = np.ascontiguousarray(np.asarray(inputs["x"], dtype=np.float32))
    w = np.ascontiguousarray(np.asarray(inputs["w"], dtype=np.float32))
    assert x.shape == (M, K) and w.shape == (N, K)

    if "nc" not in _NC_CACHE:
        _NC_CACHE["nc"] = build_nc()
    nc = _NC_CACHE["nc"]

    in_maps = []
    for c in range(8):
        mg, ng = divmod(c, NG)
        in_maps.append(
            {
                "x": x[mg * MC : (mg + 1) * MC],
                "w": w[ng * NC : (ng + 1) * NC],
            }
        )
    res = bass_utils.run_bass_kernel_spmd(nc, in_maps, core_ids=list(range(8)))

    out = np.empty((M, N), dtype=np.float32)
    for c, r in enumerate(res.results):
        mg, ng = divmod(c, NG)
        out[mg * MC : (mg + 1) * MC, ng * NC : (ng + 1) * NC] = r["out"]
    return out



# revision 7
# speedup vs baseline: 1.6472x; 1.6472x over previous
"""INT8-dynamic-quant GEMM (per-token act quant, per-channel weight quant,
int8 GEMM, dequant) on 8 trn2 NeuronCores.

Math: the int8 values (|v| <= 127) are carried exactly in bf16; bf16 matmuls
with fp32 PSUM accumulation reproduce the int32 GEMM exactly (partial sums
stay far below 2^24). Rounding uses the +1.5*2^23 magic-number trick on the
DVE (exact IEEE fp32), which is round-to-nearest-even like jnp.round.

Sharding: column-parallel over all 8 cores. Each core holds the FULL x
[8192, 4096] and an N-slice of w [1376, 4096]; it produces out[:, n-slice].

Per-core data flow (everything stays on-chip, no DRAM scratch for the
quantized tensors):
  1) w prep (once): 11 row-tiles [128, K] fp32 -> DVE abs-max/scale/round ->
     bf16 -> one-shot SBUF->SBUF xbar transpose into the RESIDENT
     wqt [128, 32, 1376] bf16 (86KB/partition). w scales -> flat DRAM row ->
     rank-1 PE matmul broadcast -> swb [128, 1376] f32.
  2) x streams: each 128-row x tile is read (gpsimd queue), quantized on
     DVE, one-shot transposed (sync queue) into a rotating xqt tile
     [128, 32, 128], consumed by exactly one GEMM iteration, and dropped.
  3) GEMM i (64 iters): 3 psum banks cover the whole n-slice (512+512+352);
     32 k-step accumulation; lhsT = xqt[:, k, :], rhs = wqt[:, k, n0:n0+wd].
     Dequant: ACT copy with per-partition scale sx, DVE multiply by swb
     (matches the reference's association), out row-block written on the
     scalar queue.
"""

import math

import numpy as np

import concourse.bass as bass
from concourse import bacc
import concourse.mybir as mybir
import concourse.tile as tile
from concourse import bass_utils

F32 = mybir.dt.float32
BF16 = mybir.dt.bfloat16
P = 128
MAGIC = 1.5 * 2**23  # fp32 round-to-nearest-even forcing constant
EPS = 1e-5

# Full problem
M, K, N = 8192, 4096, 11008
NG = 8  # column-parallel across all cores
NC = N // NG  # per-core n-slice: 1376
KT = K // P  # 32 k tiles
MT = M // P  # 64 m tiles
# w row-tiles: 1376 = 10*128 + 96 (xbar transpose needs multiples of 16 rows)
W_ROWS = [P] * 10 + [96]
WT = len(W_ROWS)
CHUNKS = [(0, 512), (512, 512), (1024, 352)]  # (n0, width) psum chunks
LOOK = 3  # x-prep lookahead (in m-tiles)


def quant_body(tc, x_ap, w_ap, out_ap):
    nc = tc.nc
    import contextlib

    est = contextlib.ExitStack()
    with est:
        dram = est.enter_context(tc.tile_pool(name="dram", bufs=1, space="DRAM"))
        stage = est.enter_context(tc.tile_pool(name="stage", bufs=3))
        qrow = est.enter_context(tc.tile_pool(name="qrow", bufs=2))
        wqtp = est.enter_context(tc.tile_pool(name="wqtp", bufs=1))
        xqtp = est.enter_context(tc.tile_pool(name="xqtp", bufs=LOOK + 1))
        small = est.enter_context(tc.tile_pool(name="small", bufs=4))
        scl = est.enter_context(tc.tile_pool(name="scl", bufs=1))
        rowp = est.enter_context(tc.tile_pool(name="rowp", bufs=1))
        obp = est.enter_context(tc.tile_pool(name="obp", bufs=2))
        psum = est.enter_context(tc.tile_pool(name="psum", bufs=6, space="PSUM"))
        pssc = est.enter_context(tc.tile_pool(name="pssc", bufs=2, space="PSUM"))

        swsc = dram.tile([NC], F32, tag="swsc")
        wqt = wqtp.tile([P, KT, NC], BF16, tag="wqt")
        sx_striped = scl.tile([P, MT], F32, tag="sx")
        sw_striped = scl.tile([P, WT], F32, tag="sw")
        swb = scl.tile([P, NC], F32, tag="swb")
        ones_row = scl.tile([1, P], F32, tag="ones")
        nc.vector.memset(ones_row, 1.0)

        def quant_tile(src_rows, scale_col, read_engine, pr=P):
            """[pr, K] fp32 rows -> bf16 int-valued tile + per-row scale."""
            tin = stage.tile([P, K], F32, tag="stage")
            read_engine.dma_start(tin[:pr, :], src_rows)
            mx = small.tile([P, 1], F32, tag="mx")
            nc.vector.tensor_reduce(
                mx[:pr], tin[:pr, :], axis=mybir.AxisListType.X,
                op=mybir.AluOpType.max, apply_absolute_value=True,
            )
            nc.vector.tensor_scalar_max(mx[:pr], mx[:pr], EPS)
            nc.vector.tensor_scalar_mul(scale_col, mx[:pr], 1.0 / 127.0)
            inv = small.tile([P, 1], F32, tag="inv")
            nc.vector.reciprocal(inv[:pr], scale_col)
            nc.vector.tensor_scalar(
                tin[:pr, :], tin[:pr, :], inv[:pr], MAGIC,
                mybir.AluOpType.mult, mybir.AluOpType.add,
            )
            q = qrow.tile([P, K], BF16, tag="qrow")
            nc.vector.tensor_scalar(
                q[:pr, :], tin[:pr, :], MAGIC, None, mybir.AluOpType.subtract,
            )
            return q

        # ---- phase W: quantize + transpose the whole w slice (resident) ----
        for t in range(WT):
            n0 = sum(W_ROWS[:t])
            pr = W_ROWS[t]
            q = quant_tile(
                w_ap[n0 : n0 + pr, :],
                sw_striped[:pr, t : t + 1],
                nc.scalar,
                pr=pr,
            )
            nc.sync.dma_start_transpose(
                wqt[:, :, n0 : n0 + pr], q[:pr, :]
            )
        # striped [128, WT] -> flat DRAM [NC] in n-order
        nc.gpsimd.dma_start(
            swsc[: 10 * P].rearrange("(o p) -> p o", p=P),
            sw_striped[:, :10],
        )
        nc.gpsimd.dma_start(
            swsc[10 * P :].rearrange("(o p) -> p o", p=96),
            sw_striped[:96, 10:11],
        )
        swr = rowp.tile([1, NC], F32, tag="swrow")
        nc.scalar.dma_start(swr[0:1, :], swsc[None, :])
        for n0, wd in CHUNKS:
            scps = pssc.tile([P, 512], F32, tag="scps")
            nc.tensor.matmul(
                scps[:, :wd], lhsT=ones_row, rhs=swr[0:1, n0 : n0 + wd],
                start=True, stop=True,
            )
            nc.scalar.copy(swb[:, n0 : n0 + wd], scps[:, :wd])

        # ---- phase X/GEMM: stream x tiles through the resident wqt ----
        def emit_x_prep(i):
            q = quant_tile(
                x_ap[i * P : (i + 1) * P, :],
                sx_striped[:, i : i + 1],
                nc.gpsimd,
            )
            xq = xqtp.tile([P, KT, P], BF16, tag="xqt", name=f"xqt{i}")
            nc.sync.dma_start_transpose(xq, q)
            return xq

        xq_tiles = {}
        for i in range(min(LOOK, MT)):
            xq_tiles[i] = emit_x_prep(i)
        for i in range(MT):
            xq = xq_tiles.pop(i)
            if i + LOOK < MT:
                xq_tiles[i + LOOK] = emit_x_prep(i + LOOK)
            pss = []
            for c, (n0, wd) in enumerate(CHUNKS):
                pss.append(psum.tile([P, 512], F32, tag="ps", name=f"ps{c}"))
            for k in range(KT):
                for c, (n0, wd) in enumerate(CHUNKS):
                    nc.tensor.matmul(
                        pss[c][:, :wd],
                        lhsT=xq[:, k, :],
                        rhs=wqt[:, k, n0 : n0 + wd],
                        start=(k == 0),
                        stop=(k == KT - 1),
                    )
            ob = obp.tile([P, NC], F32, tag="ob")
            for c, (n0, wd) in enumerate(CHUNKS):
                nc.scalar.activation(
                    ob[:, n0 : n0 + wd], pss[c][:, :wd],
                    mybir.ActivationFunctionType.Copy,
                    scale=sx_striped[:, i : i + 1],
                )
            nc.vector.tensor_tensor(ob, ob, swb, mybir.AluOpType.mult)
            nc.scalar.dma_start(out_ap[i * P : (i + 1) * P, :], ob)


def build_nc():
    nc = bacc.Bacc("TRN2", target_bir_lowering=False, debug=False, num_devices=8)
    x_d = nc.dram_tensor("x", [M, K], F32, kind="ExternalInput")
    w_d = nc.dram_tensor("w", [NC, K], F32, kind="ExternalInput")
    out_d = nc.dram_tensor("out", [M, NC], F32, kind="ExternalOutput")
    with tile.TileContext(nc) as tc:
        quant_body(tc, x_d.ap(), w_d.ap(), out_d.ap())
    nc.compile()
    return nc


_NC_CACHE = {}


def get_nc():
    if "nc" not in _NC_CACHE:
        _NC_CACHE["nc"] = build_nc()
    return _NC_CACHE["nc"]


def shard_inputs(x, w):
    return [
        {"x": x, "w": np.ascontiguousarray(w[c * NC : (c + 1) * NC])}
        for c in range(8)
    ]


def gather(results):
    out = np.empty((M, N), dtype=np.float32)
    for c, r in enumerate(results):
        out[:, c * NC : (c + 1) * NC] = r["out"]
    return out


def kernel(**inputs):
    x = np.ascontiguousarray(np.asarray(inputs["x"], dtype=np.float32))
    w = np.ascontiguousarray(np.asarray(inputs["w"], dtype=np.float32))
    assert x.shape == (M, K) and w.shape == (N, K)
    nc = get_nc()
    res = bass_utils.run_bass_kernel_spmd(
        nc, shard_inputs(x, w), core_ids=list(range(8))
    )
    return gather(res.results)
